# revision 7
# baseline (speedup 1.0000x reference)
"""Cross-layer transcoder kernel for Trainium2 (8 NeuronCores, SPMD).

Math (from the reference):
    feats[l] = relu(x[l] @ W_enc[l].T + b_enc[l])          # [B, F] per layer
    recon[j] = sum_{i<=j} feats[i] @ W_dec[i, j] + b_dec[j] # [B, D] per layer

Sharding: the transcoder feature dim F=4096 is split across the 8 cores
(512 features each). Each core encodes its feature slice for all layers and
computes a partial reconstruction for every destination layer; the partials
are summed on the host (the gather/unshard step), where b_dec and the fp8
correction terms are also added.

Precision: encode and the non-fp8 decode run fp16 with fp32 PSUM
accumulation (fp16 matmuls run at the same 1 row/cycle as bf16 but carry
10-bit mantissas, so base error is negligible). 18 of the 36 decode pairs
(i, j) run as fp8e4 DoubleRow matmuls: K=256 per instruction, so a DR pair
costs 24 matmul slots instead of 48 (each slot is 216ns on HW regardless of
dtype — measured; DR gives exactly 2x work per instruction, the cost
model's 0.5 cycles/row does not materialize).

fp8 numerics (rel-err gate is 2e-2; this config sims and measures 1.93e-2
on the reference inputs):
  - feats are centered and scaled: g = (f - 0.4375) * 32. 0.4375*32 = 14 is
    exactly representable in e4m3, so the ~50% exact-zero feats stay exact,
    and 0.4375 ~= E[f | f>0] minimizes E[(f-c)^2] (the feats-side quant
    error). The exact centering term 0.4375 * colsum(q8(W_dec)) is added on
    the host.
  - W_dec is scaled by 1024 and clipped to +-240 before e4m3 quantization;
    the psum partial is rescaled by 2^-15 on-device.
  - host mu-correction: the dominant W-side quant error component is
    E_b[feats] @ dW (dW = quantization error of W_dec, known on host).
    E_b[feats_k] is estimated analytically as E[relu(N(b_enc_k, s_k^2))]
    with s_k = ||W_enc[k,:]|| (x ~ N(0,I)), and mu_hat @ dW is subtracted
    per destination on the host. Worth ~2.7e-3 of rel err.

Scheduling notes:
  - All input tensors are packed on the host so each logical load is ONE
    contiguous 2D DMA (a dma_start costs ~610ns of queue issue time; the
    original per-chunk loads spent ~11.6us just issuing layer 0's inputs,
    delaying the first encode to ~17us).
  - j<=6 output DMAs are issued by the gpsimd sequencer (SWDGE): its
    end-of-program drain hides under decode j=7. j=7 outputs go via sync,
    which has no input loads left by then and drains fast.
  - DoubleRow matmuls run first in each psum group so the descale copy
    overlaps the fp16 matmuls instead of extending the tail.
  - x tiles are split into 512-position halves; a 28-matmul warmup covers
    the prologue DMA fill while ramping the PE clock to 2.4 GHz.
  - Output partials are written fp16 and summed in fp32 on the host.
"""

import os

import numpy as np
import ml_dtypes

L = 8          # n_layers
B = 1024       # n_pos
D = 768        # d_model
F = 4096       # d_transcoder
NCORES = 8
FL = F // NCORES   # features per core = 512
P = 128
KD = D // P        # 6  encode contraction chunks
MF = FL // P       # 4  feature chunks per core
MD = D // P        # 6  decode output chunks
NB = B // 512      # 2  position chunks of 512

# Decode pairs computed in fp8 DoubleRow; the rest run fp16.
FP8_PAIRS = (
    [(0, j) for j in range(L)]
    + [(1, j) for j in range(1, L)]
    + [(2, 5), (2, 6), (2, 7)]
)
FP8_SET = set(FP8_PAIRS)
FP8_LAYERS = sorted({i for i, _ in FP8_PAIRS})      # layers needing fp8 feats
BF_PAIRS = [
    (i, j) for j in range(L) for i in range(j + 1) if (i, j) not in FP8_SET
]
NBF = len(BF_PAIRS)     # 18
BF_IDX = {p: k for k, p in enumerate(BF_PAIRS)}

SF = 32.0               # feats fp8 scale
SW = 1024.0             # W_dec fp8 scale
CEN = 0.4375            # feats centering offset (CEN*SF = 14.0, exact in e4m3)
DESCALE = 1.0 / (SF * SW)

FP16 = np.float16
F8 = ml_dtypes.float8_e4m3

# Filled by the first kernel() call; reused afterwards.
_PROGRAM = None
# Stash of the most recent run's profiling results (test.py reads these).
LAST_EXEC_NS = None
LAST_RESULTS = None


def _build_program():
    import concourse.bacc as bacc
    import concourse.mybir as mybir
    import concourse.tile as tile

    nc = bacc.Bacc("TRN2", target_bir_lowering=False, debug=False)
    f16 = mybir.dt.float16
    f8 = mybir.dt.float8e4
    f32 = mybir.dt.float32

    # Host-packed so that each of these is ONE contiguous-per-partition DMA:
    #   xT[l, nb]  : [P, KD*512]  x transposed, kd-major along the free dim
    #   wencT[l]   : [P, KD*FL]   W_enc transposed, kd-major
    #   wdec[pair] : [P, MF*D]    fp16 decode weights, kf-major
    #   wdec8[pair]: [P, 2*2*D]   fp8 decode weights, (c, s)-major
    xT_d = nc.dram_tensor("xT", [L, NB, P, KD * 512], f16, kind="ExternalInput")
    wencT_d = nc.dram_tensor("wencT", [L, P, KD * FL], f16, kind="ExternalInput")
    benc_d = nc.dram_tensor("benc", [P, L * MF], f32, kind="ExternalInput")
    wdec_d = nc.dram_tensor("wdec", [NBF, P, MF * D], f16, kind="ExternalInput")
    wdec8_d = nc.dram_tensor(
        "wdec8", [len(FP8_PAIRS), P, 2, 2, D], f8, kind="ExternalInput"
    )
    out_d = nc.dram_tensor("outT", [L, D, B], f16, kind="ExternalOutput")

    relu = mybir.ActivationFunctionType.Relu
    mulop = mybir.AluOpType.mult
    subop = mybir.AluOpType.subtract
    dr = mybir.MatmulPerfMode.DoubleRow

    with tile.TileContext(nc) as tc:
        with (
            tc.tile_pool(name="feats", bufs=1) as feats_pool,
            tc.tile_pool(name="benc", bufs=1) as benc_pool,
            tc.tile_pool(name="xt", bufs=4) as xt_pool,
            tc.tile_pool(name="wenc", bufs=3) as wenc_pool,
            tc.tile_pool(name="wdec", bufs=7) as wdec_pool,
            tc.tile_pool(name="wdec8", bufs=6) as wdec8_pool,
            tc.tile_pool(name="outs", bufs=8) as out_pool,
            tc.tile_pool(name="psum", bufs=8, space="PSUM") as psum_pool,
        ):
            # Bias tile: one DMA for all layers' encode biases.
            bt = benc_pool.tile([P, L * MF], f32, name="benc")
            nc.sync.dma_start(bt, benc_d[:])

            # Warm up the tensor engine during the prologue DMA fill; these
            # dummy matmuls depend only on a memset tile and ramp the PE
            # clock out of its 1.2 GHz idle pstate.
            warm = feats_pool.tile([P, 512], f16, name="warm")
            nc.vector.memset(warm, 0)
            # 12 matmuls ≈ 5us at the cold 1.2 GHz clock — ends just as
            # layer 0's batched input DMAs (~2.2MB) finish landing.
            wps = psum_pool.tile([P, 512], f32, name="wps", tag="psum")
            for w in range(12):
                nc.tensor.matmul(
                    wps,
                    lhsT=warm[:, :P],
                    rhs=warm,
                    start=(w == 0),
                    stop=(w == 11),
                )

            feats = {}
            ft8 = {i: {} for i in FP8_LAYERS}
            for j in range(L):
                # ---------- encode layer j into feats[(j, mf)] ----------
                we = wenc_pool.tile([P, KD * FL], f16, name="we", tag="we")
                nc.sync.dma_start(we, wencT_d[j])
                xts = []
                for nb in range(NB):
                    xt = xt_pool.tile([P, KD * 512], f16, name="xt", tag="xt")
                    nc.sync.dma_start(xt, xT_d[j, nb])
                    xts.append(xt)
                for mf in range(MF):
                    ft = feats_pool.tile([P, B], f16, name=f"feat_{j}_{mf}")
                    feats[(j, mf)] = ft
                if j in FP8_LAYERS:
                    # Centered fp8 copy of this layer's feats for DoubleRow:
                    # ft8[j][c][p, s, b] = (feats[f=c*256+s*128+p, b]-CEN)*SF
                    for c in range(2):
                        ft8[j][c] = feats_pool.tile(
                            [P, 2, B], f8, name=f"ft8_{j}_{c}"
                        )
                for nb in range(NB):
                    for mf in range(MF):
                        ps = psum_pool.tile([P, 512], f32, name="ps", tag="psum")
                        for kd in range(KD):
                            fs = kd * FL + mf * P
                            nc.tensor.matmul(
                                ps,
                                lhsT=we[:, fs:fs + P],
                                rhs=xts[nb][:, kd * 512:(kd + 1) * 512],
                                start=(kd == 0),
                                stop=(kd == KD - 1),
                            )
                        idx = j * MF + mf
                        bsl = slice(nb * 512, (nb + 1) * 512)
                        nc.scalar.activation(
                            feats[(j, mf)][:, bsl],
                            ps,
                            relu,
                            bias=bt[:, idx:idx + 1],
                        )
                        if j in FP8_LAYERS:
                            nc.vector.tensor_scalar(
                                ft8[j][mf >> 1][:, mf & 1, bsl],
                                feats[(j, mf)][:, bsl],
                                SF,
                                CEN * SF,
                                mulop,
                                subop,
                            )

                # ---------- decode destination layer j ----------
                fp8_is = [i for i in FP8_LAYERS if (i, j) in FP8_SET]
                bf_is = [i for i in range(j + 1) if (i, j) not in FP8_SET]
                nmm = len(bf_is) * MF
                n8 = len(fp8_is) * 2
                wt8 = {}
                for i in fp8_is:
                    p8 = FP8_PAIRS.index((i, j))
                    w8 = wdec8_pool.tile([P, 2, 2, D], f8, name="wd8", tag="wd8")
                    nc.sync.dma_start(w8, wdec8_d[p8])
                    wt8[i] = w8
                wts = {}
                for i in bf_is:
                    pidx = BF_IDX[(i, j)]
                    wt = wdec_pool.tile([P, MF * D], f16, name="wd", tag="wd")
                    nc.sync.dma_start(wt, wdec_d[pidx])
                    wts[i] = wt
                # The fp16 decode weights are pre-scaled by SF*SW = 2^15 on
                # the host (exact power of 2), so fp8-DR and fp16 matmuls
                # accumulate into ONE psum group and a single x2^-15 descale
                # produces the output tile.
                ntot = n8 + nmm
                for nb in range(NB):
                    for md in range(MD):
                        dsl = slice(md * P, (md + 1) * P)
                        bsl = slice(nb * 512, (nb + 1) * 512)
                        ps = psum_pool.tile([P, 512], f32, name="ps", tag="psum")
                        cnt = 0
                        for i in fp8_is:
                            for c in range(2):
                                nc.tensor.matmul(
                                    ps,
                                    lhsT=wt8[i][:, c, :, dsl],
                                    rhs=ft8[i][c][:, :, bsl],
                                    start=(cnt == 0),
                                    stop=(cnt == ntot - 1),
                                    perf_mode=dr,
                                )
                                cnt += 1
                        for i in bf_is:
                            for kf in range(MF):
                                ds = kf * D + md * P
                                nc.tensor.matmul(
                                    ps,
                                    lhsT=wts[i][:, ds:ds + P],
                                    rhs=feats[(i, kf)][:, bsl],
                                    start=(cnt == 0),
                                    stop=(cnt == ntot - 1),
                                )
                                cnt += 1
                        ot = out_pool.tile([P, 512], f16, name="ot", tag="ot")
                        nc.vector.tensor_scalar_mul(ot, ps, DESCALE)
                        # j<=6 partials drain via the gpsimd SWDGE queue (its
                        # end-of-program drain hides under decode j=7);
                        # j=7 partials go via sync, which has no input loads
                        # left to issue by then and drains fast.
                        if j < L - 1:
                            nc.gpsimd.dma_start(out_d[j, dsl, bsl], ot)
                        else:
                            nc.sync.dma_start(out_d[j, dsl, bsl], ot)

    nc.compile()
    return nc


def _prepare_inputs(x, W_enc, b_enc, W_dec):
    """Host-side shard + pack + cast. Returns in_maps for the 8 cores."""
    # xT[l, nb, p, kd*512 + c] = x[l, nb*512 + c, kd*128 + p]
    xT = np.ascontiguousarray(
        x.reshape(L, NB, 512, KD, P).transpose(0, 1, 4, 3, 2)
        .reshape(L, NB, P, KD * 512)
    ).astype(FP16)
    in_maps = []
    for ci in range(NCORES):
        s = slice(ci * FL, (ci + 1) * FL)
        # wencT[l, p, kd*FL + f] = W_enc[l, s + f, kd*128 + p]
        wencT = np.ascontiguousarray(
            W_enc[:, s, :].reshape(L, FL, KD, P).transpose(0, 3, 2, 1)
            .reshape(L, P, KD * FL)
        ).astype(FP16)
        benc = np.ascontiguousarray(
            b_enc[:, s].reshape(L, MF, P).transpose(2, 0, 1).reshape(P, L * MF),
            dtype=np.float32,
        )
        # wdec[k, p, kf*D + d] = W_dec[i, j, s + kf*128 + p, d] * SF*SW
        # (pre-scaled by 2^15 so fp16 matmuls share the fp8 psum group and
        # its single x2^-15 descale; exact since the scale is a power of 2)
        wdec = np.empty((NBF, P, MF * D), dtype=FP16)
        for k, (i, j) in enumerate(BF_PAIRS):
            wdec[k] = (
                (W_dec[i, j, s, :] * (SF * SW))
                .reshape(MF, P, D).transpose(1, 0, 2)
                .reshape(P, MF * D)
            )
        # wdec8[k, p, c, s2, d] = q8(W_dec[i, j, s + c*256 + s2*128 + p, d])
        wdec8 = np.empty((len(FP8_PAIRS), P, 2, 2, D), dtype=F8)
        for k, (i, j) in enumerate(FP8_PAIRS):
            w = np.clip(W_dec[i, j, s, :] * SW, -240, 240)
            wdec8[k] = w.reshape(2, 2, P, D).transpose(2, 0, 1, 3).astype(F8)
        in_maps.append(
            {
                "xT": xT,
                "wencT": wencT,
                "benc": benc,
                "wdec": wdec,
                "wdec8": wdec8,
            }
        )
    return in_maps


def kernel(x, W_enc, b_enc, W_dec, b_dec):
    global _PROGRAM, LAST_EXEC_NS, LAST_RESULTS
    from concourse import bass_utils

    x = np.asarray(x)
    W_enc = np.asarray(W_enc)
    b_enc = np.asarray(b_enc)
    W_dec = np.asarray(W_dec)
    b_dec = np.asarray(b_dec)

    if _PROGRAM is None:
        _PROGRAM = _build_program()
    nc = _PROGRAM

    in_maps = _prepare_inputs(x, W_enc, b_enc, W_dec)

    # Host corrections for the fp8 pairs. Each fp8 pair computed
    #   (f_hat - CEN) @ w_hat   with w_hat = q8(w*SW)/SW,
    # so add back CEN * colsum(w_hat), and subtract the estimated
    # mean-feats component of the W quantization error, mu_hat @ (w_hat - w).
    # mu_hat_k = E[relu(N(b_enc_k, s_k^2))], s_k = ||W_enc[k,:]|| (x~N(0,I)).
    sig = np.sqrt(np.sum(W_enc.astype(np.float32) ** 2, axis=2))   # [L, F]
    t = b_enc / np.maximum(sig, 1e-9)
    Phi = 0.5 * (1.0 + _erf(t / np.sqrt(2.0)))
    phi = np.exp(-0.5 * t * t) / np.sqrt(2.0 * np.pi)
    mu_hat = b_enc * Phi + sig * phi                               # [L, F]

    corr = np.zeros((L, D), dtype=np.float32)
    for ci, m in enumerate(in_maps):
        s = slice(ci * FL, (ci + 1) * FL)
        # w8 natural order: [npair, p, c, s2, D] -> [npair, c*256+s2*128+p, D]
        w8 = m["wdec8"].astype(np.float32)
        w8nat = w8.transpose(0, 2, 3, 1, 4).reshape(len(FP8_PAIRS), FL, D)
        for k, (i, j) in enumerate(FP8_PAIRS):
            what = w8nat[k] / SW                                   # [FL, D]
            corr[j] += CEN * what.sum(axis=0)
            dw = what - W_dec[i, j, s, :]
            corr[j] -= mu_hat[i, s] @ dw

    trace = os.environ.get("KERNEL_TRACE", "0") == "1"
    res = bass_utils.run_bass_kernel_spmd(
        nc, in_maps, core_ids=list(range(NCORES)), trace=trace
    )
    LAST_EXEC_NS = res.exec_time_ns
    LAST_RESULTS = res

    acc = np.zeros((L, D, B), dtype=np.float32)
    for r in res.results:
        acc += np.asarray(r["outT"], dtype=np.float32)
    out = (
        acc.transpose(0, 2, 1)
        + b_dec.astype(np.float32)[:, None, :]
        + corr[:, None, :]
    )
    return np.ascontiguousarray(out, dtype=np.float32)


def _erf(a):
    """Vectorized erf without scipy (kernel.py must be self-contained)."""
    try:
        from scipy.special import erf as _serf
        return _serf(a)
    except Exception:
        import math
        return np.vectorize(math.erf, otypes=[np.float64])(a).astype(np.float64)


# revision 13
# speedup vs baseline: 1.0170x; 1.0170x over previous
"""Cross-layer transcoder kernel for Trainium2 (8 NeuronCores, SPMD).

Math (from the reference):
    feats[l] = relu(x[l] @ W_enc[l].T + b_enc[l])          # [B, F] per layer
    recon[j] = sum_{i<=j} feats[i] @ W_dec[i, j] + b_dec[j] # [B, D] per layer

Sharding: the transcoder feature dim F=4096 is split across the 8 cores
(512 features each). Each core encodes its feature slice for all layers and
computes a partial reconstruction for every destination layer; the partials
are summed on the host (the gather/unshard step), where b_dec and the fp8
correction terms are also added.

Precision: encode and the non-fp8 decode run fp16 with fp32 PSUM
accumulation (fp16 matmuls run at the same 1 row/cycle as bf16 but carry
10-bit mantissas, so base error is negligible). 18 of the 36 decode pairs
(i, j) run as fp8e4 DoubleRow matmuls: K=256 per instruction, so a DR pair
costs 24 matmul slots instead of 48 (each slot is 216ns on HW regardless of
dtype — measured; DR gives exactly 2x work per instruction, the cost
model's 0.5 cycles/row does not materialize).

fp8 numerics (rel-err gate is 2e-2; this config sims and measures 1.93e-2
on the reference inputs):
  - feats are centered and scaled: g = (f - 0.4375) * 32. 0.4375*32 = 14 is
    exactly representable in e4m3, so the ~50% exact-zero feats stay exact,
    and 0.4375 ~= E[f | f>0] minimizes E[(f-c)^2] (the feats-side quant
    error). The exact centering term 0.4375 * colsum(q8(W_dec)) is added on
    the host.
  - W_dec is scaled by 1024 and clipped to +-240 before e4m3 quantization;
    the psum partial is rescaled by 2^-15 on-device.
  - host mu-correction: the dominant W-side quant error component is
    E_b[feats] @ dW (dW = quantization error of W_dec, known on host).
    E_b[feats_k] is estimated analytically as E[relu(N(b_enc_k, s_k^2))]
    with s_k = ||W_enc[k,:]|| (x ~ N(0,I)), and mu_hat @ dW is subtracted
    per destination on the host. Worth ~2.7e-3 of rel err.

Scheduling notes:
  - All input tensors are packed on the host so each logical load is ONE
    contiguous 2D DMA (a dma_start costs ~610ns of queue issue time; the
    original per-chunk loads spent ~11.6us just issuing layer 0's inputs,
    delaying the first encode to ~17us).
  - j<=6 output DMAs are issued by the gpsimd sequencer (SWDGE): its
    end-of-program drain hides under decode j=7. j=7 outputs go via sync,
    which has no input loads left by then and drains fast.
  - DoubleRow matmuls run first in each psum group so the descale copy
    overlaps the fp16 matmuls instead of extending the tail.
  - x tiles are split into 512-position halves; a 28-matmul warmup covers
    the prologue DMA fill while ramping the PE clock to 2.4 GHz.
  - Output partials are written fp16 and summed in fp32 on the host.
"""

import os

import numpy as np
import ml_dtypes

L = 8          # n_layers
B = 1024       # n_pos
D = 768        # d_model
F = 4096       # d_transcoder
NCORES = 8
FL = F // NCORES   # features per core = 512
P = 128
KD = D // P        # 6  encode contraction chunks
MF = FL // P       # 4  feature chunks per core
MD = D // P        # 6  decode output chunks
NB = B // 512      # 2  position chunks of 512

# Decode pairs computed in fp8 DoubleRow; the rest run fp16.
FP8_PAIRS = (
    [(0, j) for j in range(L)]
    + [(1, j) for j in range(1, L)]
    + [(2, 4), (2, 5), (2, 6), (2, 7)]
)
FP8_SET = set(FP8_PAIRS)
FP8_LAYERS = sorted({i for i, _ in FP8_PAIRS})      # layers needing fp8 feats
BF_PAIRS = [
    (i, j) for j in range(L) for i in range(j + 1) if (i, j) not in FP8_SET
]
NBF = len(BF_PAIRS)     # 18
BF_IDX = {p: k for k, p in enumerate(BF_PAIRS)}

SF = 32.0               # feats fp8 scale
SW = 1024.0             # W_dec fp8 scale
CEN = 0.4375            # feats centering offset (CEN*SF = 14.0, exact in e4m3)
DESCALE = 1.0 / (SF * SW)

FP16 = np.float16
F8 = ml_dtypes.float8_e4m3

# Filled by the first kernel() call; reused afterwards.
_PROGRAM = None
# Stash of the most recent run's profiling results (test.py reads these).
LAST_EXEC_NS = None
LAST_RESULTS = None


def _build_program():
    import concourse.bacc as bacc
    import concourse.mybir as mybir
    import concourse.tile as tile

    nc = bacc.Bacc("TRN2", target_bir_lowering=False, debug=False)
    f16 = mybir.dt.float16
    f8 = mybir.dt.float8e4
    f32 = mybir.dt.float32

    # Host-packed so that each of these is ONE contiguous-per-partition DMA:
    #   xT[l, nb]  : [P, KD*512]  x transposed, kd-major along the free dim
    #   wencT[l]   : [P, KD*FL]   W_enc transposed, kd-major
    #   wdec[pair] : [P, MF*D]    fp16 decode weights, kf-major
    #   wdec8[pair]: [P, 2*2*D]   fp8 decode weights, (c, s)-major
    xT_d = nc.dram_tensor("xT", [L, NB, P, KD * 512], f16, kind="ExternalInput")
    wencT_d = nc.dram_tensor("wencT", [L, P, KD * FL], f16, kind="ExternalInput")
    benc_d = nc.dram_tensor("benc", [P, L * MF], f32, kind="ExternalInput")
    wdec_d = nc.dram_tensor("wdec", [NBF, P, MF * D], f16, kind="ExternalInput")
    wdec8_d = nc.dram_tensor(
        "wdec8", [len(FP8_PAIRS), P, 2, 2, D], f8, kind="ExternalInput"
    )
    out_d = nc.dram_tensor("outT", [L, D, B], f16, kind="ExternalOutput")
    fsum_d = nc.dram_tensor(
        "fsum", [P, len(FP8_LAYERS) * MF], f32, kind="ExternalOutput"
    )

    relu = mybir.ActivationFunctionType.Relu
    mulop = mybir.AluOpType.mult
    subop = mybir.AluOpType.subtract
    dr = mybir.MatmulPerfMode.DoubleRow

    with tile.TileContext(nc) as tc:
        with (
            tc.tile_pool(name="feats", bufs=1) as feats_pool,
            tc.tile_pool(name="benc", bufs=1) as benc_pool,
            tc.tile_pool(name="xt", bufs=4) as xt_pool,
            tc.tile_pool(name="wenc", bufs=3) as wenc_pool,
            tc.tile_pool(name="wdec", bufs=7) as wdec_pool,
            tc.tile_pool(name="wdec8", bufs=6) as wdec8_pool,
            tc.tile_pool(name="outs", bufs=8) as out_pool,
            tc.tile_pool(name="psum", bufs=8, space="PSUM") as psum_pool,
        ):
            # Bias tile: one DMA for all layers' encode biases.
            bt = benc_pool.tile([P, L * MF], f32, name="benc")
            nc.sync.dma_start(bt, benc_d[:])

            # Warm up the tensor engine during the prologue DMA fill; these
            # dummy matmuls depend only on a memset tile and ramp the PE
            # clock out of its 1.2 GHz idle pstate.
            warm = feats_pool.tile([P, 512], f16, name="warm")
            nc.vector.memset(warm, 0)
            wps = psum_pool.tile([P, 512], f32, name="wps", tag="psum")
            for w in range(28):
                nc.tensor.matmul(
                    wps,
                    lhsT=warm[:, :P],
                    rhs=warm,
                    start=(w == 0),
                    stop=(w == 27),
                )

            # Batch-sums of the fp8 source layers' feats (vector-engine
            # reduces, off the critical path). The host turns these into the
            # exact mean-feats correction mu @ dW for the fp8 weight
            # quantization error.
            fsum = benc_pool.tile([P, len(FP8_LAYERS) * MF], f32, name="fsum")

            feats = {}
            ft8 = {i: {} for i in FP8_LAYERS}
            for j in range(L):
                # ---------- encode layer j into feats[(j, mf)] ----------
                we = wenc_pool.tile([P, KD * FL], f16, name="we", tag="we")
                nc.sync.dma_start(we, wencT_d[j])
                xts = []
                for nb in range(NB):
                    xt = xt_pool.tile([P, KD * 512], f16, name="xt", tag="xt")
                    nc.sync.dma_start(xt, xT_d[j, nb])
                    xts.append(xt)
                for mf in range(MF):
                    ft = feats_pool.tile([P, B], f16, name=f"feat_{j}_{mf}")
                    feats[(j, mf)] = ft
                if j in FP8_LAYERS:
                    # Centered fp8 copy of this layer's feats for DoubleRow:
                    # ft8[j][c][p, s, b] = (feats[f=c*256+s*128+p, b]-CEN)*SF
                    for c in range(2):
                        ft8[j][c] = feats_pool.tile(
                            [P, 2, B], f8, name=f"ft8_{j}_{c}"
                        )
                for nb in range(NB):
                    for mf in range(MF):
                        ps = psum_pool.tile([P, 512], f32, name="ps", tag="psum")
                        for kd in range(KD):
                            fs = kd * FL + mf * P
                            nc.tensor.matmul(
                                ps,
                                lhsT=we[:, fs:fs + P],
                                rhs=xts[nb][:, kd * 512:(kd + 1) * 512],
                                start=(kd == 0),
                                stop=(kd == KD - 1),
                            )
                        idx = j * MF + mf
                        bsl = slice(nb * 512, (nb + 1) * 512)
                        nc.scalar.activation(
                            feats[(j, mf)][:, bsl],
                            ps,
                            relu,
                            bias=bt[:, idx:idx + 1],
                        )
                        if j in FP8_LAYERS:
                            nc.vector.tensor_scalar(
                                ft8[j][mf >> 1][:, mf & 1, bsl],
                                feats[(j, mf)][:, bsl],
                                SF,
                                CEN * SF,
                                mulop,
                                subop,
                            )
                if j in FP8_LAYERS:
                    li = FP8_LAYERS.index(j)
                    for mf in range(MF):
                        fi = li * MF + mf
                        nc.vector.reduce_sum(
                            fsum[:, fi:fi + 1],
                            feats[(j, mf)],
                            axis=mybir.AxisListType.X,
                        )

                # ---------- decode destination layer j ----------
                fp8_is = [i for i in FP8_LAYERS if (i, j) in FP8_SET]
                bf_is = [i for i in range(j + 1) if (i, j) not in FP8_SET]
                nmm = len(bf_is) * MF
                n8 = len(fp8_is) * 2
                wt8 = {}
                for i in fp8_is:
                    p8 = FP8_PAIRS.index((i, j))
                    w8 = wdec8_pool.tile([P, 2, 2, D], f8, name="wd8", tag="wd8")
                    nc.sync.dma_start(w8, wdec8_d[p8])
                    wt8[i] = w8
                wts = {}
                for i in bf_is:
                    pidx = BF_IDX[(i, j)]
                    wt = wdec_pool.tile([P, MF * D], f16, name="wd", tag="wd")
                    nc.sync.dma_start(wt, wdec_d[pidx])
                    wts[i] = wt
                # The fp16 decode weights are pre-scaled by SF*SW = 2^15 on
                # the host (exact power of 2), so fp8-DR and fp16 matmuls
                # accumulate into ONE psum group and a single x2^-15 descale
                # produces the output tile.
                ntot = n8 + nmm
                for nb in range(NB):
                    for md in range(MD):
                        dsl = slice(md * P, (md + 1) * P)
                        bsl = slice(nb * 512, (nb + 1) * 512)
                        ps = psum_pool.tile([P, 512], f32, name="ps", tag="psum")
                        cnt = 0
                        for i in fp8_is:
                            for c in range(2):
                                nc.tensor.matmul(
                                    ps,
                                    lhsT=wt8[i][:, c, :, dsl],
                                    rhs=ft8[i][c][:, :, bsl],
                                    start=(cnt == 0),
                                    stop=(cnt == ntot - 1),
                                    perf_mode=dr,
                                )
                                cnt += 1
                        for i in bf_is:
                            for kf in range(MF):
                                ds = kf * D + md * P
                                nc.tensor.matmul(
                                    ps,
                                    lhsT=wts[i][:, ds:ds + P],
                                    rhs=feats[(i, kf)][:, bsl],
                                    start=(cnt == 0),
                                    stop=(cnt == ntot - 1),
                                )
                                cnt += 1
                        ot = out_pool.tile([P, 512], f16, name="ot", tag="ot")
                        nc.vector.tensor_scalar_mul(ot, ps, DESCALE)
                        # j<=6 partials drain via the gpsimd SWDGE queue (its
                        # end-of-program drain hides under decode j=7);
                        # j=7 partials go via sync, which has no input loads
                        # left to issue by then and drains fast.
                        if j < L - 1:
                            nc.gpsimd.dma_start(out_d[j, dsl, bsl], ot)
                        else:
                            nc.sync.dma_start(out_d[j, dsl, bsl], ot)

            nc.sync.dma_start(fsum_d[:], fsum)

    nc.compile()
    return nc


def _prepare_inputs(x, W_enc, b_enc, W_dec):
    """Host-side shard + pack + cast. Returns in_maps for the 8 cores."""
    # xT[l, nb, p, kd*512 + c] = x[l, nb*512 + c, kd*128 + p]
    xT = np.ascontiguousarray(
        x.reshape(L, NB, 512, KD, P).transpose(0, 1, 4, 3, 2)
        .reshape(L, NB, P, KD * 512)
    ).astype(FP16)
    in_maps = []
    for ci in range(NCORES):
        s = slice(ci * FL, (ci + 1) * FL)
        # wencT[l, p, kd*FL + f] = W_enc[l, s + f, kd*128 + p]
        wencT = np.ascontiguousarray(
            W_enc[:, s, :].reshape(L, FL, KD, P).transpose(0, 3, 2, 1)
            .reshape(L, P, KD * FL)
        ).astype(FP16)
        benc = np.ascontiguousarray(
            b_enc[:, s].reshape(L, MF, P).transpose(2, 0, 1).reshape(P, L * MF),
            dtype=np.float32,
        )
        # wdec[k, p, kf*D + d] = W_dec[i, j, s + kf*128 + p, d] * SF*SW
        # (pre-scaled by 2^15 so fp16 matmuls share the fp8 psum group and
        # its single x2^-15 descale; exact since the scale is a power of 2)
        wdec = np.empty((NBF, P, MF * D), dtype=FP16)
        for k, (i, j) in enumerate(BF_PAIRS):
            wdec[k] = (
                (W_dec[i, j, s, :] * (SF * SW))
                .reshape(MF, P, D).transpose(1, 0, 2)
                .reshape(P, MF * D)
            )
        # wdec8[k, p, c, s2, d] = q8(W_dec[i, j, s + c*256 + s2*128 + p, d])
        wdec8 = np.empty((len(FP8_PAIRS), P, 2, 2, D), dtype=F8)
        for k, (i, j) in enumerate(FP8_PAIRS):
            w = np.clip(W_dec[i, j, s, :] * SW, -240, 240)
            wdec8[k] = w.reshape(2, 2, P, D).transpose(2, 0, 1, 3).astype(F8)
        in_maps.append(
            {
                "xT": xT,
                "wencT": wencT,
                "benc": benc,
                "wdec": wdec,
                "wdec8": wdec8,
            }
        )
    return in_maps


def kernel(x, W_enc, b_enc, W_dec, b_dec):
    global _PROGRAM, LAST_EXEC_NS, LAST_RESULTS
    from concourse import bass_utils

    x = np.asarray(x)
    W_enc = np.asarray(W_enc)
    b_enc = np.asarray(b_enc)
    W_dec = np.asarray(W_dec)
    b_dec = np.asarray(b_dec)

    if _PROGRAM is None:
        _PROGRAM = _build_program()
    nc = _PROGRAM

    in_maps = _prepare_inputs(x, W_enc, b_enc, W_dec)

    trace = os.environ.get("KERNEL_TRACE", "0") == "1"
    res = bass_utils.run_bass_kernel_spmd(
        nc, in_maps, core_ids=list(range(NCORES)), trace=trace
    )
    LAST_EXEC_NS = res.exec_time_ns
    LAST_RESULTS = res

    # Host corrections for the fp8 pairs. Each fp8 pair computed
    #   (f_hat - CEN) @ w_hat   with w_hat = q8(w*SW)/SW,
    # so add back CEN * colsum(w_hat), and subtract the exact mean-feats
    # component of the W quantization error, mu @ (w_hat - w), using the
    # device-computed batch sums of the fp8 source layers' feats.
    corr = np.zeros((L, D), dtype=np.float32)
    acc = np.zeros((L, D, B), dtype=np.float32)
    for ci, (m, r) in enumerate(zip(in_maps, res.results)):
        acc += np.asarray(r["outT"], dtype=np.float32)
        s = slice(ci * FL, (ci + 1) * FL)
        # mu[li, mf*128 + p] = fsum[p, li*MF + mf] / B
        fsum = np.asarray(r["fsum"], dtype=np.float32)             # [P, 3*MF]
        mu = (
            fsum.reshape(P, len(FP8_LAYERS), MF).transpose(1, 2, 0)
            .reshape(len(FP8_LAYERS), FL)
            / B
        )
        # w8 natural order: [npair, p, c, s2, D] -> [npair, c*256+s2*128+p, D]
        w8 = m["wdec8"].astype(np.float32)
        w8nat = w8.transpose(0, 2, 3, 1, 4).reshape(len(FP8_PAIRS), FL, D)
        for k, (i, j) in enumerate(FP8_PAIRS):
            what = w8nat[k] / SW                                   # [FL, D]
            corr[j] += CEN * what.sum(axis=0)
            dw = what - W_dec[i, j, s, :]
            corr[j] -= mu[FP8_LAYERS.index(i)] @ dw

    out = (
        acc.transpose(0, 2, 1)
        + b_dec.astype(np.float32)[:, None, :]
        + corr[:, None, :]
    )
    return np.ascontiguousarray(out, dtype=np.float32)
